# revision 6
# baseline (speedup 1.0000x reference)
"""Trainium2 kernel for nn_EnhancedHybridModel (hybrid MLP + 8-qubit circuit).

Reformulation (exact, up to f32 rounding):
  * BatchNorms are inference-mode -> folded into the adjacent Linear.
  * The quantum circuit after RY-encoding uses shared weights, so it is one
    fixed complex matrix M (256x256).  The encoded state is a REAL product
    vector s[b] = kron_i [cos(pre_i/2), -sin(pre_i/2)].
  * q_out = |M s|^2 @ Z  ->  y = [Re M; Im M] @ s  (512x256 matmul),
    then q_out @ W4eff.T folds with the Z-projection into M4 (512x32):
    h4 = relu(y^2 @ M4 + b4eff).

Data parallel over 8 NeuronCores: batch 65536 -> 8192 rows/core.
Per-core layout: activations kept as [features, batch_cols] (batch on the
free axis, 16 column-tiles of 512).  The product-state build happens in
[batch, state] layout (cheap broadcast krons on DVE) and is transposed back
with PE-transposes.  All matmuls run as float32r (full-rate fp32).
"""

import numpy as np

import concourse.bass as bass
import concourse.mybir as mybir
import concourse.tile as tile
from concourse import bacc
from concourse.bass_utils import run_bass_kernel_spmd
from concourse.masks import make_identity

F32 = mybir.dt.float32
F32R = mybir.dt.float32r
AF = mybir.ActivationFunctionType
ALU = mybir.AluOpType

N_CORES = 8
BATCH = 65536
B_CORE = BATCH // N_CORES  # 8192
COLS = 512  # batch columns per tile (one PSUM bank of f32)
NTILES = B_CORE // COLS  # 16

N_QUBITS = 8
N_LAYERS = 3
DIM = 256
EPS = 1e-5

# ---------------------------------------------------------------- host math

_idx = np.arange(DIM)
_CNOT_PERMS = []
for _i in range(N_QUBITS):
    for _j in range(_i + 1, N_QUBITS):
        _c = (_idx >> (N_QUBITS - 1 - _i)) & 1
        _CNOT_PERMS.append(np.where(_c == 1, _idx ^ (1 << (N_QUBITS - 1 - _j)), _idx))
_Z_SIGNS = np.stack(
    [1.0 - 2.0 * ((_idx >> (N_QUBITS - 1 - i)) & 1) for i in range(N_QUBITS)], axis=1
).astype(np.float64)


def _rx(t):
    c, s = np.cos(t / 2), -1j * np.sin(t / 2)
    return np.array([[c, s], [s, c]], np.complex128)


def _ry(t):
    c, s = np.cos(t / 2), np.sin(t / 2)
    return np.array([[c, -s], [s, c]], np.complex128)


def _rz(t):
    e = np.exp(-0.5j * t)
    return np.array([[e, 0], [0, np.conj(e)]], np.complex128)


def _apply_gate(M, G, w):
    # reference einsum('st,bpsq->bptq', U, state): state'[t] = sum_s U[s,t] state[s]
    left = 2**w
    Mr = M.reshape(left, 2, -1, DIM)
    return np.einsum("st,psqj->ptqj", G, Mr).reshape(DIM, DIM)


def _build_circuit_matrix(q_weights):
    qw = np.asarray(q_weights, np.float64)
    M = np.eye(DIM, dtype=np.complex128)
    for l in range(N_LAYERS):
        for i in range(N_QUBITS):
            M = _apply_gate(M, _rx(qw[l, i, 0]), i)
            M = _apply_gate(M, _ry(qw[l, i, 1]), i)
            M = _apply_gate(M, _rz(qw[l, i, 2]), i)
        for perm in _CNOT_PERMS:
            M = M[perm, :]
    return M


def _fold_bn(W, b, g, bt, m, v):
    sc = np.asarray(g, np.float64) / np.sqrt(np.asarray(v, np.float64) + EPS)
    Weff = sc[:, None] * np.asarray(W, np.float64)
    beff = (np.asarray(b, np.float64) - np.asarray(m, np.float64)) * sc + np.asarray(
        bt, np.float64
    )
    return Weff, beff


def _prep_consts(inputs):
    f = {k: np.asarray(v, np.float64) for k, v in inputs.items() if k != "x"}
    W1e, b1e = _fold_bn(f["W1"], f["b1"], f["g1"], f["bt1"], f["m1"], f["v1"])
    W2e, b2e = _fold_bn(f["W2"], f["b2"], f["g2"], f["bt2"], f["m2"], f["v2"])
    W4e, b4e = _fold_bn(f["W4"], f["b4"], f["g4"], f["bt4"], f["m4"], f["v4"])
    M = _build_circuit_matrix(f["q_weights"])
    C = np.concatenate([M.real, M.imag], axis=0)  # (512, 256)
    Zst = np.concatenate([_Z_SIGNS, _Z_SIGNS], axis=0)  # (512, 8)
    M4 = Zst @ W4e.T  # (512, 32)

    def col(v, p):  # bias as a [p, 1] column
        return np.ascontiguousarray(np.asarray(v, np.float64).reshape(p, 1)).astype(
            np.float32
        )

    return {
        "W1T": np.ascontiguousarray(W1e.T).astype(np.float32),  # (16,128)
        "W2T": np.ascontiguousarray(W2e.T).astype(np.float32),  # (128,64)
        "W3T": np.ascontiguousarray(f["W3"].T).astype(np.float32),  # (64,8)
        "CT": np.ascontiguousarray(C.T).astype(np.float32),  # (256,512)
        "M4": np.ascontiguousarray(M4).astype(np.float32),  # (512,32)
        "W5T": np.ascontiguousarray(f["W5"].T).astype(np.float32),  # (32,16)
        "W6T": np.ascontiguousarray(f["W6"].T).astype(np.float32),  # (16,1)
        "b1": col(b1e, 128),
        "b2": col(b2e, 64),
        "b3": col(f["b3"], 8),
        "b4": col(b4e, 32),
        "b5": col(f["b5"], 16),
        "b6": col(f["b6"], 1),
    }


# ------------------------------------------------------------- bass program


def _ap(t, offset, dims):
    """Custom free-dim access pattern on a tile: keep its partition dim."""
    a = t[:]
    return bass.AP(a.tensor, a.offset + offset, [list(a.ap[0])] + [list(d) for d in dims])


def _build_nc():
    nc = bacc.Bacc("TRN2", target_bir_lowering=False, debug=False)

    xt = nc.dram_tensor("xt", [16, B_CORE], F32, kind="ExternalInput")
    ct_d = nc.dram_tensor("CT", [256, 512], F32, kind="ExternalInput")
    m4_d = nc.dram_tensor("M4", [512, 32], F32, kind="ExternalInput")
    w1_d = nc.dram_tensor("W1T", [16, 128], F32, kind="ExternalInput")
    w2_d = nc.dram_tensor("W2T", [128, 64], F32, kind="ExternalInput")
    w3_d = nc.dram_tensor("W3T", [64, 8], F32, kind="ExternalInput")
    w5_d = nc.dram_tensor("W5T", [32, 16], F32, kind="ExternalInput")
    w6_d = nc.dram_tensor("W6T", [16, 1], F32, kind="ExternalInput")
    b_d = {
        name: nc.dram_tensor(name, [p, 1], F32, kind="ExternalInput")
        for name, p in [("b1", 128), ("b2", 64), ("b3", 8), ("b4", 32), ("b5", 16), ("b6", 1)]
    }
    out_d = nc.dram_tensor("out", [1, B_CORE], F32, kind="ExternalOutput")

    HALF_PI = float(np.pi / 2)

    with tile.TileContext(nc) as tc:
        with (
            tc.tile_pool(name="const", bufs=1) as cp,
            tc.tile_pool(name="work", bufs=3) as wp,
            tc.tile_pool(name="pmlp", bufs=2, space="PSUM") as pmlp,
            tc.tile_pool(name="ptr", bufs=3, space="PSUM") as ptr,
            tc.tile_pool(name="py", bufs=3, space="PSUM") as py,
        ):
            ident = cp.tile([128, 128], F32)
            make_identity(nc, ident[:])
            half_pi = cp.tile([128, 1], F32)
            nc.gpsimd.memset(half_pi[:], HALF_PI)

            ct_f = cp.tile([128, 1024], F32)
            nc.sync.dma_start(ct_f[:, 0:512], ct_d[0:128, :])
            nc.sync.dma_start(ct_f[:, 512:1024], ct_d[128:256, :])
            m4_f = cp.tile([128, 128], F32)
            for c in range(4):
                nc.sync.dma_start(m4_f[:, 32 * c : 32 * (c + 1)], m4_d[128 * c : 128 * (c + 1), :])
            w1_f = cp.tile([16, 128], F32)
            nc.sync.dma_start(w1_f[:], w1_d[:])
            w2_f = cp.tile([128, 64], F32)
            nc.sync.dma_start(w2_f[:], w2_d[:])
            w3_f = cp.tile([64, 8], F32)
            nc.sync.dma_start(w3_f[:], w3_d[:])
            w5_f = cp.tile([32, 16], F32)
            nc.sync.dma_start(w5_f[:], w5_d[:])
            w6_f = cp.tile([16, 1], F32)
            nc.sync.dma_start(w6_f[:], w6_d[:])
            # one-time f32 -> f32r rounding copies (walrus requires f32r
            # matmul operands to be produced as f32r)
            ct = cp.tile([128, 1024], F32R)
            nc.vector.tensor_copy(ct[:], ct_f[:])
            m4 = cp.tile([128, 128], F32R)
            nc.vector.tensor_copy(m4[:], m4_f[:])
            w1 = cp.tile([16, 128], F32R)
            nc.vector.tensor_copy(w1[:], w1_f[:])
            w2 = cp.tile([128, 64], F32R)
            nc.vector.tensor_copy(w2[:], w2_f[:])
            w3 = cp.tile([64, 8], F32R)
            nc.vector.tensor_copy(w3[:], w3_f[:])
            w5 = cp.tile([32, 16], F32R)
            nc.vector.tensor_copy(w5[:], w5_f[:])
            w6 = cp.tile([16, 1], F32R)
            nc.vector.tensor_copy(w6[:], w6_f[:])
            bias = {}
            for name, p in [("b1", 128), ("b2", 64), ("b3", 8), ("b4", 32), ("b5", 16), ("b6", 1)]:
                bias[name] = cp.tile([p, 1], F32, name=name, tag=name)
                nc.sync.dma_start(bias[name][:], b_d[name][:])

            def mm(out, lhsT, rhs, start=True, stop=True):
                nc.tensor.matmul(out, lhsT, rhs, start=start, stop=stop)

            for t in range(NTILES):
                cols = slice(COLS * t, COLS * (t + 1))

                x_f = wp.tile([16, COLS], F32, tag="xf")
                nc.sync.dma_start(x_f[:], xt[:, cols])
                x_t = wp.tile([16, COLS], F32R, tag="x")
                nc.scalar.activation(x_t[:], x_f[:], AF.Copy)

                h1p = pmlp.tile([128, COLS], F32, tag="mlp")
                mm(h1p[:], w1[:], x_t[:])
                h1 = wp.tile([128, COLS], F32R, tag="h1")
                nc.scalar.activation(h1[:], h1p[:], AF.Relu, bias=bias["b1"][:])

                h2p = pmlp.tile([64, COLS], F32, tag="mlp")
                mm(h2p[:], w2[:], h1[:])
                h2 = wp.tile([64, COLS], F32R, tag="h2")
                nc.vector.tensor_scalar(h2[:], h2p[:], bias["b2"][:], 0.0, ALU.add, ALU.max)

                prp = pmlp.tile([8, COLS], F32, tag="mlp")
                mm(prp[:], w3[:], h2[:])
                pre = wp.tile([8, COLS], F32, tag="pre")
                nc.scalar.activation(pre[:], prp[:], AF.Tanh, bias=bias["b3"][:])

                # transpose pre -> [128 batch, 8 angles] per 128-col block
                preT = ptr.tile([128, 32], F32, tag="tr")
                for b in range(4):
                    nc.tensor.transpose(
                        preT[:, 8 * b : 8 * (b + 1)],
                        pre[:, 128 * b : 128 * (b + 1)],
                        ident[0:8, 0:8],
                    )
                # cs[:, 16b + q] = cos(pre_q/2), cs[:, 16b + 8 + q] = -sin(pre_q/2)
                cs = wp.tile([128, 64], F32, tag="cs")
                pin = _ap(preT, 0, [[8, 4], [1, 8]])
                nc.scalar.activation(_ap(cs, 0, [[16, 4], [1, 8]]), pin, AF.Sin, bias=half_pi[:], scale=0.5)
                nc.scalar.activation(_ap(cs, 8, [[16, 4], [1, 8]]), pin, AF.Sin, bias=0.0, scale=-0.5)

                sT0 = wp.tile([128, COLS], F32R, tag="sT0")
                sT1 = wp.tile([128, COLS], F32R, tag="sT1")
                qp = wp.tile([128, 64], F32, tag="qp")
                uv = wp.tile([128, 128], F32, tag="uv")
                for b in range(4):
                    # level 1: qp[4p + 2a + b'] = cs[8a+2p] * cs[8b'+2p+1]
                    nc.vector.tensor_mul(
                        _ap(qp, 16 * b, [[4, 4], [2, 2], [1, 2]]),
                        _ap(cs, 16 * b, [[2, 4], [8, 2], [0, 2]]),
                        _ap(cs, 16 * b + 1, [[2, 4], [0, 2], [8, 2]]),
                    )
                    # level 2: uv[16h + 4i + j] = qp[8h+i] * qp[8h+4+j]
                    nc.vector.tensor_mul(
                        _ap(uv, 32 * b, [[16, 2], [4, 4], [1, 4]]),
                        _ap(qp, 16 * b, [[8, 2], [1, 4], [0, 4]]),
                        _ap(qp, 16 * b + 4, [[8, 2], [0, 4], [1, 4]]),
                    )
                    # level 3: sB[16a + b'] = uv[a] * uv[16+b']
                    sB = wp.tile([128, 256], F32, tag="sB")
                    nc.vector.tensor_mul(
                        _ap(sB, 0, [[16, 16], [1, 16]]),
                        _ap(uv, 32 * b, [[1, 16], [0, 16]]),
                        _ap(uv, 32 * b + 16, [[0, 16], [1, 16]]),
                    )
                    # transpose to [state_k, batch] chunks
                    tp0 = ptr.tile([128, 128], F32, tag="tr")
                    nc.tensor.transpose(tp0[:], sB[:, 0:128], ident[:])
                    tp1 = ptr.tile([128, 128], F32, tag="tr")
                    nc.tensor.transpose(tp1[:], sB[:, 128:256], ident[:])
                    dst0 = sT0[:, 128 * b : 128 * (b + 1)]
                    dst1 = sT1[:, 128 * b : 128 * (b + 1)]
                    if b % 2 == 0:
                        nc.scalar.activation(dst0, tp0[:], AF.Copy)
                        nc.vector.tensor_copy(dst1, tp1[:])
                    else:
                        nc.vector.tensor_copy(dst0, tp0[:])
                        nc.scalar.activation(dst1, tp1[:], AF.Copy)

                # y = C @ s ; sq = y^2
                sq = []
                for mc in range(4):
                    yp = py.tile([128, COLS], F32, tag="y")
                    mm(yp[:], ct[:, 128 * mc : 128 * (mc + 1)], sT0[:], start=True, stop=False)
                    mm(yp[:], ct[:, 512 + 128 * mc : 512 + 128 * (mc + 1)], sT1[:], start=False, stop=True)
                    sq_mc = wp.tile([128, COLS], F32R, tag=f"sq{mc}")
                    nc.scalar.activation(sq_mc[:], yp[:], AF.Square)
                    sq.append(sq_mc)

                h4p = pmlp.tile([32, COLS], F32, tag="mlp")
                for mc in range(4):
                    mm(h4p[:], m4[:, 32 * mc : 32 * (mc + 1)], sq[mc][:], start=(mc == 0), stop=(mc == 3))
                h4 = wp.tile([32, COLS], F32R, tag="h4")
                nc.scalar.activation(h4[:], h4p[:], AF.Relu, bias=bias["b4"][:])

                h5p = pmlp.tile([16, COLS], F32, tag="mlp")
                mm(h5p[:], w5[:], h4[:])
                h5 = wp.tile([16, COLS], F32R, tag="h5")
                nc.vector.tensor_scalar(h5[:], h5p[:], bias["b5"][:], 0.0, ALU.add, ALU.max)

                op = pmlp.tile([1, COLS], F32, tag="mlp")
                mm(op[:], w6[:], h5[:])
                o_sb = wp.tile([1, COLS], F32, tag="o")
                nc.vector.tensor_scalar_add(o_sb[:], op[:], bias["b6"][:])
                nc.sync.dma_start(out_d[0:1, cols], o_sb[:])

    nc.compile()
    return nc


_NC_CACHE = []

# test-harness hooks (unused in grading): set _TRACE to profile; the full
# BassKernelResults of the last run lands in _LAST_RESULTS[0].
_TRACE = False
_LAST_RESULTS = []


def _get_nc():
    if not _NC_CACHE:
        _NC_CACHE.append(_build_nc())
    return _NC_CACHE[0]


def kernel(**inputs):
    consts = _prep_consts(inputs)
    x = np.ascontiguousarray(np.asarray(inputs["x"], np.float32))  # (65536, 16)
    xt_full = np.ascontiguousarray(x.T)  # (16, 65536)

    nc = _get_nc()
    in_maps = []
    for c in range(N_CORES):
        m = {"xt": np.ascontiguousarray(xt_full[:, c * B_CORE : (c + 1) * B_CORE])}
        m.update(consts)
        in_maps.append(m)
    res = run_bass_kernel_spmd(nc, in_maps, list(range(N_CORES)), trace=_TRACE)
    _LAST_RESULTS.clear()
    _LAST_RESULTS.append(res)
    out = np.concatenate([r["out"].reshape(B_CORE) for r in res.results])
    return out.reshape(BATCH, 1).astype(np.float32)
